# revision 1
# baseline (speedup 1.0000x reference)
"""Trainium2 Bass kernel for nn_ConvKernelBank.

Computation:
  alpha = softmax_M( causal_conv1d( gelu(pre_w @ mean_hw(q) + pre_b), mix_w ) + mix_b )
  k_out = sum_m alpha[b,m,t] * dwconv3d_causal(k, Wk[m])
  v_out = sum_m alpha[b,m,t] * dwconv3d_causal(v, Wv[m])

Strategy:
  - 8 NeuronCores, data-parallel over (batch, T-half): core i handles
    b = i // 2, t in [16*(i%2), 16*(i%2)+16).  Causal temporal halo of 2
    frames is passed in from the host (zeros at sequence start).
  - Layout [C=128 partitions, (t, h, w) free]; k/v arrive spatially
    zero-padded to 26x26 per frame so every conv tap reads a full
    24x24 window (pad supplies the conv zero-padding).
  - The mixture weights alpha[m, t] are folded into per-output-frame
    effective 27-tap depthwise filters W_eff[c, tap] (3 convs + mix ->
    one conv).
  - The 27-tap MAC work is split across engines:
      * PE (TensorE): most frames.  Each tap becomes a matmul with a
        diagonal [128,128] weight matrix (diag = per-channel tap
        weights), accumulated over the 27 taps in PSUM, in float32r
        (1 cycle/row when the moving free dim >= 256).  Frames are
        processed as two 12-row halves so a PSUM tile fits one bank.
      * DVE: remaining frames via scalar_tensor_tensor MACs.
    Diagonal weight matrices are built from an identity matrix scaled
    by W_eff columns, spread across ACT / DVE / Pool(gpsimd).  ACT also
    drains PSUM to SBUF for the output DMA.
"""

import os
from contextlib import ExitStack

import numpy as np

import concourse.bass as bass
import concourse.tile as tile
from concourse import mybir
from concourse.bass_utils import run_bass_kernel_spmd
from concourse.vector_clock import ScopedClock

B, C, T, H, W = 4, 128, 32, 24, 24
M, KT, KS, MIXK = 3, 3, 3, 3
HW = H * W
NCORES = 8
TLOC = 16          # output frames per core
THALO = 2          # causal temporal halo
TIN = TLOC + THALO
F32 = mybir.dt.float32
F32R = mybir.dt.float32r
F16 = mybir.dt.float16
NTAP = KT * KS * KS  # 27
WBLK = 2 * NTAP      # 54: 27 k-taps then 27 v-taps per frame
PADS = KS - 1        # spatial padding (1 on each side)
PH, PW = H + PADS, W + PADS          # 26, 26
PHW = PH * PW                        # 676
HHALF = H // 2                       # 12 rows per PSUM half
FHALF = HHALF * W                    # 288 free elements per half

AluOp = mybir.AluOpType
ActFn = mybir.ActivationFunctionType

# frame-tensors handled by DVE stt MACs (the rest go to PE):
# (t, xi) with xi: 0=k, 1=v
DVE_FT = {(2, 0), (2, 1), (6, 0), (6, 1), (10, 0), (10, 1), (14, 0), (14, 1)}
# how many of the 27 diag builds per PE frame-tensor go to each engine
DIAG_DVE = 0
DIAG_POOL = 10  # built with broadcast tensor_tensor on gpsimd
DIAG_LOOKAHEAD = 1  # PE frame-tensors of diag pre-build before matmuls start


class _SplitDrainTileContext(tile.TileContext):
    """TileContext whose final drain splits semaphore waits across several
    drain instructions: this walrus build rejects >2 sync waits on one
    CTRL instruction ("Too many sync wait commands")."""

    MAX_WAITS = 1

    def _drain_and_barrier(self, tick_clock, wait_clock):
        nc = self.nc
        drain_inst = nc.sync.drain()
        wait_clock.add_sem_waits(
            drain_inst.ins, ScopedClock({None: tick_clock.global_clock})
        )
        mi = drain_inst.ins
        si = mi.sync_info
        waits = list(si.on_wait or []) if si is not None else []
        if len(waits) > self.MAX_WAITS:
            si.on_wait = waits[: self.MAX_WAITS]
            rest = waits[self.MAX_WAITS :]
            for i in range(0, len(rest), self.MAX_WAITS):
                d2 = nc.sync.drain()
                d2.ins.sync_info = mybir.SyncInfo(
                    on_wait=rest[i : i + self.MAX_WAITS], on_update=[]
                )
        nc.all_engine_barrier()
        popped = nc._tile_sem_poison_stack.pop()
        assert popped is self._sem_poison
        nc.clear_and_free_semaphores(list(self.sems.allocated().values()))
        nc.all_engine_barrier()


_MAX_SYNC_WAITS = 1

_NOP_ENGINES = {
    mybir.EngineType.PE,
    mybir.EngineType.DVE,
    mybir.EngineType.Activation,
    mybir.EngineType.Pool,
    mybir.EngineType.SP,
}


def _split_sync_waits(nc: bass.Bass, max_waits: int = _MAX_SYNC_WAITS) -> None:
    """Walrus rejects instructions carrying more than ~2 semaphore waits.
    Move excess waits onto freshly inserted same-engine NoOps placed just
    before the offending instruction (waiting earlier is always safe)."""
    for fn in nc.m.functions:
        for bb in fn.blocks:
            new_list = []
            changed = False
            for inst in bb.instructions:
                si = inst.sync_info
                waits = list(si.on_wait) if (si is not None and si.on_wait) else []
                if len(waits) > max_waits:
                    assert inst.engine in _NOP_ENGINES, (
                        f"can't split waits on {inst.engine} {type(inst).__name__}"
                    )
                    excess, keep = waits[:-max_waits], waits[-max_waits:]
                    for i in range(0, len(excess), max_waits):
                        nop = mybir.InstNoOp(
                            name=nc.get_next_instruction_name(), ins=[], outs=[]
                        )
                        nop.engine = inst.engine
                        nop.sync_info = mybir.SyncInfo(
                            on_wait=excess[i : i + max_waits], on_update=[]
                        )
                        new_list.append(nop)
                    si.on_wait = keep
                    changed = True
                new_list.append(inst)
            if changed:
                bb.instructions[:] = new_list


def _build_program() -> bass.Bass:
    nc = bass.Bass()

    qin = nc.declare_dram_parameter("qin", [C, TIN * HW], F16, isOutput=False)
    # spatially padded (26x26 per frame), fp32r so the fp32r matmuls may
    # consume it (bass/walrus requires fp32r producers; bit-identical data)
    kin = nc.declare_dram_parameter("kin", [C, TIN * PHW], F16, isOutput=False)
    vin = nc.declare_dram_parameter("vin", [C, TIN * PHW], F16, isOutput=False)
    # [c, m*54 + tap] : tap 0..26 = Wk[m,c], 27..53 = Wv[m,c]
    wkv = nc.declare_dram_parameter("wkv", [C, M * WBLK], F32, isOutput=False)
    # (pre_w / HW).T  -> lhsT layout [c_in, c_out]
    prew = nc.declare_dram_parameter("prew", [C, C], F32, isOutput=False)
    preb = nc.declare_dram_parameter("preb", [C, 1], F32, isOutput=False)
    # [c, j*3 + m] = mix_w[m, c, j]
    mixw = nc.declare_dram_parameter("mixw", [C, MIXK * M], F32, isOutput=False)
    # mix_b tiled to [TLOC, M]
    mixb = nc.declare_dram_parameter("mixb", [TLOC, M], F32, isOutput=False)
    # halo validity mask (0 for the 2 halo columns on sequence-start cores)
    hmask = nc.declare_dram_parameter("hmask", [C, THALO], F32, isOutput=False)
    ident = nc.declare_dram_parameter("ident", [C, C], F16, isOutput=False)

    kout = nc.declare_dram_parameter("kout", [C, TLOC * HW], F32, isOutput=True)
    vout = nc.declare_dram_parameter("vout", [C, TLOC * HW], F32, isOutput=True)

    # scratch DRAM for the alpha partition-broadcast round trip
    adram = nc.dram_tensor("alpha_rt", [TLOC * M], F32)

    with ExitStack() as ctx:
        tc = ctx.enter_context(_SplitDrainTileContext(nc))

        consts = ctx.enter_context(tc.tile_pool(name="consts", bufs=1))
        big = ctx.enter_context(tc.tile_pool(name="big", bufs=1))
        small = ctx.enter_context(tc.tile_pool(name="small", bufs=1))
        diagp = ctx.enter_context(tc.tile_pool(name="diagp", bufs=58))
        outp = ctx.enter_context(tc.tile_pool(name="outp", bufs=6))
        stg = ctx.enter_context(tc.tile_pool(name="stg", bufs=8))
        apsum = ctx.enter_context(tc.tile_pool(name="apsum", bufs=1, space="PSUM"))
        cpsum = ctx.enter_context(tc.tile_pool(name="cpsum", bufs=5, space="PSUM"))

        # ---- load constants ----
        wkv_sb = consts.tile([C, M * WBLK], F32)
        nc.sync.dma_start(wkv_sb[:], wkv[:])
        prew_sb = consts.tile([C, C], F32)
        nc.sync.dma_start(prew_sb[:], prew[:])
        preb_sb = consts.tile([C, 1], F32)
        nc.sync.dma_start(preb_sb[:], preb[:])
        mixw_sb = consts.tile([C, MIXK * M], F32)
        nc.sync.dma_start(mixw_sb[:], mixw[:])
        mixb_sb = consts.tile([TLOC, M], F32)
        nc.sync.dma_start(mixb_sb[:], mixb[:])
        hmask_sb = consts.tile([C, THALO], F32)
        nc.sync.dma_start(hmask_sb[:], hmask[:])
        id_sb = consts.tile([C, C], F16)
        nc.sync.dma_start(id_sb[:], ident[:])

        # ---- load bulk data ----
        # Separate chunk tiles so consumers only wait on the chunk they read,
        # split across the two HWDGE queues (SP + ACT) to stream in parallel.
        # SP queue: q0, v0 now; v1, v2 after the alpha round-trip. ACT queue:
        # q1, k0, k1, k2.
        QCH = 9   # q frames per chunk
        XCH = 6   # k/v frames per chunk
        q_ch = [big.tile([C, QCH * HW], F16, name=f"q{i}") for i in range(2)]
        nc.sync.dma_start(q_ch[0][:], qin[:, : QCH * HW])
        nc.scalar.dma_start(q_ch[1][:], qin[:, QCH * HW :])
        k_ch = [big.tile([C, XCH * PHW], F16, name=f"k{i}") for i in range(3)]
        v_ch = [big.tile([C, XCH * PHW], F16, name=f"v{i}") for i in range(3)]
        for i in range(3):
            nc.scalar.dma_start(k_ch[i][:], kin[:, i * XCH * PHW : (i + 1) * XCH * PHW])
        nc.sync.dma_start(v_ch[0][:], vin[:, : XCH * PHW])

        # ---- mix predictor ----
        # qg[c, t] = sum_hw q (1/HW folded into prew on host).  Pooling split
        # between ACT (accum_out) and DVE (tensor_reduce) to shorten the
        # serial head.
        qg = small.tile([C, TIN], F32)
        qscratch = small.tile([C, HW], F32)
        qscratch2 = small.tile([C, HW], F16)
        for t in range(TIN):
            src = q_ch[t // QCH][:, (t % QCH) * HW : (t % QCH + 1) * HW]
            if t % 3 == 0:
                nc.scalar.activation(
                    qscratch[:], src, ActFn.Copy, accum_out=qg[:, t : t + 1]
                )
            else:
                # tensor_scalar gets DVE 2x mode (tensor_reduce does not);
                # accum_out needs an explicit 2nd op for the reduce
                nc.vector.tensor_scalar(
                    qscratch2[:], src, 1.0, 0.0, AluOp.mult, AluOp.add,
                    accum_out=qg[:, t : t + 1],
                )
        # h = gelu(prew.T @ qg + preb)
        h_ps = apsum.tile([C, TIN], F32)
        nc.tensor.matmul(h_ps[:], prew_sb[:], qg[:], start=True, stop=True)
        h_sb = small.tile([C, TIN], F32)
        nc.scalar.activation(
            h_sb[:], h_ps[:], ActFn.Gelu, bias=preb_sb[:, 0:1], scale=1.0
        )
        # zero the causal halo columns where the reference zero-pads h
        nc.vector.tensor_mul(h_sb[:, 0:THALO], h_sb[:, 0:THALO], hmask_sb[:])

        # logits[t, m] = sum_j sum_c mix_w[m,c,j] h[c, t+j]  (t local)
        lg_ps = apsum.tile([TLOC, M], F32)
        for j in range(MIXK):
            nc.tensor.matmul(
                lg_ps[:],
                h_sb[:, j : j + TLOC],
                mixw_sb[:, j * M : (j + 1) * M],
                start=(j == 0),
                stop=(j == MIXK - 1),
            )
        lt = small.tile([TLOC, M], F32)
        nc.vector.tensor_add(lt[:], lg_ps[:], mixb_sb[:])

        # softmax over m (free dim)
        rmax = small.tile([TLOC, 1], F32)
        nc.vector.tensor_reduce(rmax[:], lt[:], axis=mybir.AxisListType.X, op=AluOp.max)
        nmax = small.tile([TLOC, 1], F32)
        nc.vector.tensor_scalar(nmax[:], rmax[:], -1.0, None, AluOp.mult)
        ex = small.tile([TLOC, M], F32)
        nc.scalar.activation(ex[:], lt[:], ActFn.Exp, bias=nmax[:, 0:1], scale=1.0)
        ssum = small.tile([TLOC, 1], F32)
        nc.vector.tensor_reduce(ssum[:], ex[:], axis=mybir.AxisListType.X, op=AluOp.add)
        rcp = small.tile([TLOC, 1], F32)
        nc.vector.reciprocal(rcp[:], ssum[:])
        alpha_t = small.tile([TLOC, M], F32)
        nc.vector.tensor_scalar(alpha_t[:], ex[:], rcp[:, 0:1], None, AluOp.mult)

        # broadcast alpha to all 128 partitions: SBUF -> DRAM -> [1,48], then
        # PE outer product ones[128] x alpha[48] (K=1 matmul) -> PSUM -> SBUF
        nc.sync.dma_start(adram[:], alpha_t[:])
        a1 = small.tile([1, TLOC * M], F32)
        nc.sync.dma_start(a1[:], adram[:])
        # remaining v chunks, queued behind the alpha round-trip on SP
        nc.sync.dma_start(v_ch[1][:], vin[:, XCH * PHW : 2 * XCH * PHW])
        nc.sync.dma_start(v_ch[2][:], vin[:, 2 * XCH * PHW :])
        ones = small.tile([1, C], F32)
        nc.vector.memset(ones[:], 1.0)
        abc_ps = apsum.tile([C, TLOC * M], F32)
        nc.tensor.matmul(abc_ps[:], ones[:], a1[:], start=True, stop=True)
        abc = small.tile([C, TLOC * M], F32)
        nc.vector.tensor_copy(abc[:], abc_ps[:])

        # ---- fold alpha into per-frame effective filters ----
        # weff[c, t*54 + tap] = sum_m alpha[t, m] * wkv[c, m*54 + tap]
        weff = big.tile([C, TLOC * WBLK], F32)
        weff16 = big.tile([C, TLOC * WBLK], F16)
        for t in range(TLOC):
            dst = weff[:, t * WBLK : (t + 1) * WBLK]
            for m in range(M):
                a_sc = abc[:, t * M + m : t * M + m + 1]
                src = wkv_sb[:, m * WBLK : (m + 1) * WBLK]
                if m == 0:
                    nc.vector.tensor_scalar(dst, src, a_sc, None, AluOp.mult)
                else:
                    nc.vector.scalar_tensor_tensor(
                        dst, src, a_sc, dst, AluOp.mult, AluOp.add
                    )
            nc.vector.tensor_copy(
                weff16[:, t * WBLK : (t + 1) * WBLK], dst
            )

        # ---- the depthwise conv ----
        def xframe(xi, i):
            """Padded input frame i of tensor xi as [C, PH, PW] (fp16)."""
            ch = (k_ch, v_ch)[xi][i // XCH]
            return ch[:].rearrange("p (t h w) -> p t h w", t=XCH, h=PH)[:, i % XCH]

        def wcol(t, base, tap):
            c0 = t * WBLK + base + tap
            return weff[:, c0 : c0 + 1]

        # DVE frame-tensor: 27 chained scalar_tensor_tensor MACs
        def dve_frame(t, xi):
            odram = (kout, vout)[xi]
            base = xi * NTAP
            of = outp.tile([C, HW], F32, tag="of")
            o3 = of[:].rearrange("p (h w) -> p h w", h=H)
            first = True
            for dt in range(KT):
                xf = xframe(xi, t + dt)
                for dh in range(KS):
                    for dw in range(KS):
                        tap = dt * 9 + dh * 3 + dw
                        src = xf[:, dh : dh + H, dw : dw + W]
                        if first:
                            nc.vector.tensor_scalar(
                                o3[:, :, :], src, wcol(t, base, tap), None, AluOp.mult
                            )
                            first = False
                        else:
                            nc.vector.scalar_tensor_tensor(
                                o3[:, :, :], src, wcol(t, base, tap),
                                o3[:, :, :], AluOp.mult, AluOp.add,
                            )
            nc.sync.dma_start(odram[:, t * HW : (t + 1) * HW], of[:])

        # diag builds for one PE frame-tensor, spread over ACT/DVE/Pool.
        def build_diags(t, xi):
            base = xi * NTAP
            tiles = []
            for tap in range(NTAP):
                dg = diagp.tile([C, C], F16, tag="dg")
                w = wcol(t, base, tap)
                if tap < DIAG_POOL:
                    c0 = t * WBLK + base + tap
                    w16 = weff16[:, c0 : c0 + 1]
                    nc.gpsimd.tensor_tensor(
                        dg[:], id_sb[:], w16.broadcast_to([C, C]), AluOp.mult
                    )
                elif tap < DIAG_POOL + DIAG_DVE:
                    nc.vector.tensor_scalar(dg[:], id_sb[:], w, None, AluOp.mult)
                else:
                    nc.scalar.activation(dg[:], id_sb[:], ActFn.Copy, scale=w)
                tiles.append(dg)
            return tiles

        # PE frame-tensor: 2 halves x 27 accumulating diag matmuls, ACT drain
        def pe_frame(t, xi, diags):
            odram = (kout, vout)[xi]
            for half in range(2):
                h0 = half * HHALF
                acc = cpsum.tile([C, FHALF], F32, tag="pe", padded_shape=[C, 512])
                for dt in range(KT):
                    xr = xframe(xi, t + dt)
                    for dh in range(KS):
                        for dw in range(KS):
                            tap = dt * 9 + dh * 3 + dw
                            rhs = xr[:, h0 + dh : h0 + dh + HHALF, dw : dw + W]
                            nc.tensor.matmul(
                                acc[:], diags[tap][:], rhs,
                                start=(tap == 0), stop=(tap == NTAP - 1),
                            )
                st = stg.tile([C, FHALF], F32, tag="st")
                nc.scalar.activation(st[:], acc[:], ActFn.Copy)
                nc.sync.dma_start(
                    odram[:, t * HW + h0 * W : t * HW + h0 * W + FHALF], st[:]
                )

        # schedule: PE frames pipelined with a diag-build lookahead so ACT
        # stays ahead of the PE; DVE frames interleaved independently.
        order = [(t, xi) for t in range(TLOC) for xi in range(2)]
        pe_list = [ft for ft in order if ft not in DVE_FT]
        dve_list = [ft for ft in order if ft in DVE_FT]

        diag_store = {}
        for i in range(DIAG_LOOKAHEAD):
            diag_store[pe_list[i]] = build_diags(*pe_list[i])

        di = 0  # next dve frame to emit
        for i, ft in enumerate(pe_list):
            # interleave DVE frames at matching pace
            while di * len(pe_list) <= i * len(dve_list):
                dve_frame(*dve_list[di])
                di += 1
            ahead = i + DIAG_LOOKAHEAD
            if ahead < len(pe_list):
                diag_store[pe_list[ahead]] = build_diags(*pe_list[ahead])
            pe_frame(*ft, diag_store.pop(ft))
        while di < len(dve_list):
            dve_frame(*dve_list[di])
            di += 1

    _split_sync_waits(nc)
    return nc


_PROGRAM_CACHE: bass.Bass | None = None

# Results of the last hardware run (for the test harness to inspect).
LAST_RESULT = None


def _get_program() -> bass.Bass:
    global _PROGRAM_CACHE
    if _PROGRAM_CACHE is None:
        _PROGRAM_CACHE = _build_program()
    return _PROGRAM_CACHE


def _halo_pad_slice(x_b: np.ndarray, t0: int) -> np.ndarray:
    """x_b: [C, T, H, W] -> [C, TIN*PHW] fp16: 2 leading halo frames (zeros
    when t0 == 0) and each frame zero-padded spatially to 26x26."""
    out = np.zeros((C, TIN, PH, PW), dtype=np.float16)
    if t0 == 0:
        out[:, THALO:, 1 : 1 + H, 1 : 1 + W] = x_b[:, t0 : t0 + TLOC]
    else:
        out[:, :, 1 : 1 + H, 1 : 1 + W] = x_b[:, t0 - THALO : t0 + TLOC]
    return np.ascontiguousarray(out.reshape(C, TIN * PHW))


def _halo_slice(x_b: np.ndarray, t0: int) -> np.ndarray:
    if t0 == 0:
        halo = np.zeros((C, THALO, H, W), dtype=np.float16)
    else:
        halo = x_b[:, t0 - THALO : t0].astype(np.float16)
    out = np.concatenate([halo, x_b[:, t0 : t0 + TLOC].astype(np.float16)], axis=1)
    return np.ascontiguousarray(out.reshape(C, TIN * HW))


def _make_in_maps(q, k, v, Wk, Wv, pre_w, pre_b, mix_w, mix_b):
    q = np.asarray(q, dtype=np.float32)
    k = np.asarray(k, dtype=np.float32)
    v = np.asarray(v, dtype=np.float32)
    Wk = np.asarray(Wk, dtype=np.float32)
    Wv = np.asarray(Wv, dtype=np.float32)
    pre_w = np.asarray(pre_w, dtype=np.float32)
    pre_b = np.asarray(pre_b, dtype=np.float32)
    mix_w = np.asarray(mix_w, dtype=np.float32)
    mix_b = np.asarray(mix_b, dtype=np.float32)

    # shared (replicated) weight prep
    wk_flat = Wk.reshape(M, C, NTAP)  # [m, c, tap]
    wv_flat = Wv.reshape(M, C, NTAP)
    wkv_host = np.empty((C, M * WBLK), dtype=np.float32)
    for m in range(M):
        wkv_host[:, m * WBLK : m * WBLK + NTAP] = wk_flat[m].reshape(C, NTAP)
        wkv_host[:, m * WBLK + NTAP : (m + 1) * WBLK] = wv_flat[m].reshape(C, NTAP)
    prew_host = np.ascontiguousarray((pre_w / HW).T)  # [c_in, c_out]
    preb_host = np.ascontiguousarray(pre_b.reshape(C, 1))
    mixw_host = np.empty((C, MIXK * M), dtype=np.float32)
    for j in range(MIXK):
        for m in range(M):
            mixw_host[:, j * M + m] = mix_w[m, :, j]
    mixb_host = np.ascontiguousarray(np.tile(mix_b[None, :], (TLOC, 1)))
    ident_host = np.eye(C, dtype=np.float16)

    in_maps = []
    for core in range(NCORES):
        b, th = core // 2, core % 2
        t0 = th * TLOC
        hm = np.zeros((C, THALO), np.float32) if t0 == 0 else np.ones((C, THALO), np.float32)
        in_maps.append(
            {
                "qin": _halo_slice(q[b], t0),
                "kin": _halo_pad_slice(k[b], t0),
                "vin": _halo_pad_slice(v[b], t0),
                "wkv": wkv_host,
                "prew": prew_host,
                "preb": preb_host,
                "mixw": mixw_host,
                "mixb": mixb_host,
                "hmask": hm,
                "ident": ident_host,
            }
        )
    return in_maps


def kernel(q, k, v, Wk, Wv, pre_w, pre_b, mix_w, mix_b):
    in_maps = _make_in_maps(q, k, v, Wk, Wv, pre_w, pre_b, mix_w, mix_b)
    nc = _get_program()
    trace = bool(int(os.environ.get("BASSK_TRACE", "0")))
    res = run_bass_kernel_spmd(nc, in_maps, list(range(NCORES)), trace=trace)
    global LAST_RESULT
    LAST_RESULT = res

    k_out = np.empty((B, C, T, H, W), dtype=np.float32)
    v_out = np.empty((B, C, T, H, W), dtype=np.float32)
    for core in range(NCORES):
        b, th = core // 2, core % 2
        t0 = th * TLOC
        k_out[b, :, t0 : t0 + TLOC] = res.results[core]["kout"].reshape(C, TLOC, H, W)
        v_out[b, :, t0 : t0 + TLOC] = res.results[core]["vout"].reshape(C, TLOC, H, W)
    return (k_out, v_out)

